# revision 1
# baseline (speedup 1.0000x reference)
"""Trainium2 Bass kernel for nn_AttentionBlock (B=8, S=2048, D=512, f32).

Strategy: data-parallel over batch — one batch element per NeuronCore (8 cores,
same NEFF, SPMD). Per core, the full attention block is computed with the
"transposed scores" layout so no on-chip transposes are needed:

  host prep:  xt = x[b].T               [D, S]   (contiguous)
              wq = (Wq * 1/sqrt(D)).T   [D, D]   (scale folded into Wq)
              wk = Wk.T, wv = Wv.T      [D, D]
  stage A:    kT[a, s] = sum_d wk[d, a] * xt[d, s]     (PSUM accum over d)
              qT[a, s] = sum_d wq[d, a] * xt[d, s]
              v[s, e]  = sum_d xt[d, s] * wv[d, e]     (natural [S, D] layout)
  stage B:    sT[k, q] = sum_a kT[a, k] * qT[a, q]     (scores, transposed)
              p[k, q]  = exp(sT)        -- no max subtraction: scores ∈ [-10, 10]
              l[q]     = sum_k p[k, q]  (DVE partial sums + one ones-column matmul)
  stage C:    outT[e, q] = sum_k v[k, e] * p[k, q]
              outT *= 1/l  (reciprocal + K=1 broadcast matmul)
  host post:  out[b] = outT.T

Matmuls run as float32r (fp32 storage, single-pass reduced-precision PE mode —
measured ~229 ns per 128x128x512, same rate as bf16, ~4e-4 end-to-end error).
Softmax skips max-subtraction: scaled scores for this problem stay within ±10
(exp <= 2.2e4, safely inside the fp32 envelope), which is mathematically
identical to the max-subtracted softmax.

Emission order is tuned so the PE never waits: warmup matmuls ramp the HAM
clock while inputs DMA in; stage A runs s-chunk-major so the first matmuls
only need wk + the first xt chunk; the v-projection fills the gap between
scores(qc=0) and PV(qc=0); the denominator/reciprocal chain is placed so its
DVE work overlaps PV/scores matmuls.
"""

import math

import numpy as np

import concourse.mybir as mybir
import concourse.tile as tile
from concourse import bacc
from concourse.bass_utils import run_bass_kernel_spmd

P = 128          # partitions
S = 2048         # sequence length
DM = 512         # d_model == d_attn == d_value
ND = DM // P     # 4  d-model chunks
NS = S // P      # 16 sequence blocks
QC = 512         # q-chunk width for fused score/PV stages
NQC = S // QC    # 4
NEC = DM // P    # 4  e-chunks of the output
N_WARMUP = 128   # PE warmup matmuls issued while input DMAs stream

F32 = mybir.dt.float32
F32R = mybir.dt.float32r
BF16 = mybir.dt.bfloat16

# 'f32r' (default): f32 storage, float32r matmuls.  'bf16': bf16 storage+matmuls.
MODE = "f32r"

_NC_CACHE = {}


def _build(mode):
    # tensors feeding the tensor engine carry the matmul dtype: the BIR
    # verifier requires fp32r matmul operands to be *produced* as float32r
    sb_dt = BF16 if mode == "bf16" else F32R
    aux_dt = F32 if mode == "bf16" else F32R
    nc = bacc.Bacc()

    xt_d = nc.dram_tensor("xt", [DM, S], sb_dt, kind="ExternalInput")
    wq_d = nc.dram_tensor("wq", [DM, DM], sb_dt, kind="ExternalInput")
    wk_d = nc.dram_tensor("wk", [DM, DM], sb_dt, kind="ExternalInput")
    wv_d = nc.dram_tensor("wv", [DM, DM], sb_dt, kind="ExternalInput")
    outT_d = nc.dram_tensor("outT", [DM, S], F32, kind="ExternalOutput")

    mm = nc.tensor.matmul

    # f32r outputs on DVE ops trip the low-precision guard; actual matmul
    # accumulation stays in fp32 PSUM throughout.
    with nc.allow_low_precision(reason="fp32r operand rounding; PSUM accumulation is fp32"), \
         tile.TileContext(nc) as tc:
        with tc.tile_pool(name="consts", bufs=1) as consts:
            # persistent SBUF tensors (distinct tags so nothing shares slots)
            wq_sb = [consts.tile([P, DM], sb_dt, name=f"wq{i}", tag=f"wq{i}") for i in range(ND)]
            wk_sb = [consts.tile([P, DM], sb_dt, name=f"wk{i}", tag=f"wk{i}") for i in range(ND)]
            wv_sb = [consts.tile([P, DM], sb_dt, name=f"wv{i}", tag=f"wv{i}") for i in range(ND)]
            xt_sb = [consts.tile([P, S], sb_dt, name=f"xt{i}", tag=f"xt{i}") for i in range(ND)]
            kt_sb = [consts.tile([P, S], sb_dt, name=f"kt{j}", tag=f"kt{j}") for j in range(ND)]
            qt_sb = [consts.tile([P, S], sb_dt, name=f"qt{j}", tag=f"qt{j}") for j in range(ND)]
            v_sb = [consts.tile([P, DM], sb_dt, name=f"v{b}", tag=f"v{b}") for b in range(NS)]
            ones_col = consts.tile([P, 1], aux_dt, name="ones_col", tag="ones_col")
            ones_row = consts.tile([1, P], aux_dt, name="ones_row", tag="ones_row")
            # fp32 ones used by the warmup matmuls (fp32r forbids free-dim-1
            # matmuls) and, in f32r mode, as the pre-rounding memset source
            # (memset can't write f32r)
            warm_src = consts.tile([P, 1], F32, name="warm_src", tag="warm_src")
            nc.vector.memset(warm_src, 1.0)
            # preload the ACT Exp table during stage A — otherwise the first
            # exp of the scores stage pays the ~1.3us table load inline
            exp_warm = consts.tile([P, 1], F32, name="exp_warm", tag="exp_warm")
            nc.scalar.activation(out=exp_warm, in_=warm_src,
                                 func=mybir.ActivationFunctionType.Exp)
            if aux_dt == F32:
                nc.vector.memset(ones_col, 1.0)
                nc.vector.memset(ones_row, 1.0)
            else:
                ones_row_raw = consts.tile([1, P], F32, name="ones_row_raw", tag="ones_row_raw")
                nc.vector.memset(ones_row_raw, 1.0)
                nc.vector.tensor_copy(ones_col, warm_src)
                nc.vector.tensor_copy(ones_row, ones_row_raw)

            # input DMAs in first-use order: the first kT psum group needs
            # only wk's j0 columns + the first xt chunk, so those go first
            for i in range(ND):
                nc.sync.dma_start(out=wk_sb[i][:, 0:P], in_=wk_d[i * P:(i + 1) * P, 0:P])
            for i in range(ND):
                nc.sync.dma_start(out=xt_sb[i][:, 0:QC], in_=xt_d[i * P:(i + 1) * P, 0:QC])
            for i in range(ND):
                nc.sync.dma_start(out=wk_sb[i][:, P:DM], in_=wk_d[i * P:(i + 1) * P, P:DM])
            for sc in range(1, NQC):
                for i in range(ND):
                    nc.sync.dma_start(
                        out=xt_sb[i][:, sc * QC:(sc + 1) * QC],
                        in_=xt_d[i * P:(i + 1) * P, sc * QC:(sc + 1) * QC],
                    )
            for i in range(ND):
                nc.sync.dma_start(out=wq_sb[i], in_=wq_d[i * P:(i + 1) * P, :])
            for i in range(ND):
                nc.sync.dma_start(out=wv_sb[i], in_=wv_d[i * P:(i + 1) * P, :])

            # ---- stage A: k/q projections (s-chunk-major: the first groups
            # only need wk + the first xt chunk) -----------------------------
            # psA takes 4 banks and is released before psO opens; psS and psM
            # are opened while psA is still live so they get the never-used
            # banks 4-7 and carry NO dependency on psA's release (a pool
            # release waits on ALL of the pool's accessors, which otherwise
            # stalls the first scores matmul behind the last stage-A copy)
            from contextlib import ExitStack as _ExitStack
            with (
                tc.tile_pool(name="psS", bufs=3, space="PSUM") as psS,
                tc.tile_pool(name="psM", bufs=1, space="PSUM") as psM,
            ):
                _psa_stack = _ExitStack()
                psA = _psa_stack.enter_context(tc.tile_pool(name="psA", bufs=4, space="PSUM"))
                # PE warmup: tiny matmuls with no data deps keep the PE busy
                # while inputs stream in, so the HAM clock is at 2.4 GHz when
                # real matmuls start.  (shares the psA tag/slots)
                warm = psA.tile([1, 1], F32, name="warm", tag="psA")
                for w in range(N_WARMUP):
                    mm(warm, warm_src, warm_src, start=True, stop=True)
                # kT copies on ACT, qT copies on DVE: both engines are idle
                # here, and keeping ACT clear means the first exp of the
                # scores stage isn't queued behind stage-A copies
                for w_sb, t_sb, copy_op in (
                    (wk_sb, kt_sb, nc.scalar.copy),
                    (wq_sb, qt_sb, nc.vector.tensor_copy),
                ):
                    for sc in range(NQC):
                        for j in range(ND):
                            ps = psA.tile([P, QC], F32, name="psA", tag="psA")
                            for i in range(ND):
                                mm(ps, w_sb[i][:, j * P:(j + 1) * P],
                                   xt_sb[i][:, sc * QC:(sc + 1) * QC],
                                   start=(i == 0), stop=(i == ND - 1))
                            copy_op(t_sb[j][:, sc * QC:(sc + 1) * QC], ps)
                _psa_stack.close()

                # ---- stages B+C: scores -> exp -> denominators -> PV ------
                with (
                    tc.tile_pool(name="ptp", bufs=1) as ptp,
                    tc.tile_pool(name="work", bufs=2) as work,
                    tc.tile_pool(name="outp", bufs=3) as outp,
                    tc.tile_pool(name="psO", bufs=4, space="PSUM") as psO,
                ):
                    for qc in range(NQC):
                        qs = slice(qc * QC, (qc + 1) * QC)
                        pt = ptp.tile([P, NS, QC], sb_dt, name="pt", tag="pt")
                        # partial k-sums of p, computed on the (otherwise idle)
                        # DVE in four quarters so the last one lands just after
                        # the scores finish and the combined sum is ready when
                        # the l1 matmul reads it mid-PV
                        h1 = work.tile([P, QC, 1], aux_dt, name="h1", tag="h1", bufs=1)
                        h2 = work.tile([P, QC, 1], aux_dt, name="h2", tag="h2", bufs=1)
                        NQ4 = NS // 4
                        for kb in range(NS):
                            ps_s = psS.tile([P, QC], F32, name="ps_s", tag="ps_s")
                            for j in range(ND):
                                mm(ps_s, kt_sb[j][:, kb * P:(kb + 1) * P], qt_sb[j][:, qs],
                                   start=(j == 0), stop=(j == ND - 1))
                            nc.scalar.activation(out=pt[:, kb, :], in_=ps_s,
                                                 func=mybir.ActivationFunctionType.Exp)
                            if kb == NQ4 - 1:
                                nc.vector.reduce_sum(
                                    out=h1, in_=pt[:, 0:NQ4, :].rearrange("p b q -> p q b"),
                                    axis=mybir.AxisListType.X)
                            elif kb == 2 * NQ4 - 1:
                                nc.vector.reduce_sum(
                                    out=h2, in_=pt[:, NQ4:2 * NQ4, :].rearrange("p b q -> p q b"),
                                    axis=mybir.AxisListType.X)
                                nc.vector.tensor_add(h1[:, :, 0], h1[:, :, 0], h2[:, :, 0])
                            elif kb == 3 * NQ4 - 1:
                                nc.vector.reduce_sum(
                                    out=h2, in_=pt[:, 2 * NQ4:3 * NQ4, :].rearrange("p b q -> p q b"),
                                    axis=mybir.AxisListType.X)
                                nc.vector.tensor_add(h1[:, :, 0], h1[:, :, 0], h2[:, :, 0])

                        if qc == 0:
                            # v-projection, emitted here so it fills the PE while
                            # the qc=0 exps finish (PV(0) depends on all of them)
                            for b in range(NS):
                                psv = psO.tile([P, DM], F32, name="psv", tag="ps_o")
                                for i in range(ND):
                                    mm(psv, xt_sb[i][:, b * P:(b + 1) * P], wv_sb[i],
                                       start=(i == 0), stop=(i == ND - 1))
                                # scalar engine: the DVE is busy with the
                                # denominator reduces here, and Tile's static
                                # schedule would run those first, starving PV(0)
                                nc.scalar.copy(v_sb[b], psv)

                        nc.vector.reduce_sum(
                            out=h2, in_=pt[:, 3 * NQ4:NS, :].rearrange("p b q -> p q b"),
                            axis=mybir.AxisListType.X)
                        nc.vector.tensor_add(h1[:, :, 0], h1[:, :, 0], h2[:, :, 0])

                        # PV: outT[e, q] = sum_k v[k, e] * p[k, q].  The l/1/l
                        # chain (l1 matmul -> DVE reciprocal -> K=1 broadcast
                        # matmul) is threaded through the PV groups so each step's
                        # input is ready just before the PE reaches it.
                        ps_os = []
                        l1 = psM.tile([1, QC], F32, name="l1", tag="lr")
                        r_sb = work.tile([1, QC], aux_dt, name="r_sb", tag="r_sb")
                        ps_r = psM.tile([P, QC], F32, name="ps_r", tag="lr")
                        r_bc = work.tile([P, QC], F32, name="r_bc", tag="r_bc")
                        last = qc == NQC - 1
                        # l -> 1/l -> broadcast threaded two PV groups before the
                        # end, and norms emitted as soon as the broadcast lands:
                        # the DVE is then clear before the next chunk's reduces
                        l1_after, psr_after = NEC - 3, NEC - 2
                        for ec in range(NEC):
                            if last and ec == NEC - 1:
                                # final output tile in two column halves (separate
                                # PSUM banks — a shared bank would serialize on the
                                # first half's norm read): the first half's
                                # norm+DMA overlap the second half's matmuls
                                for h in range(2):
                                    hs = slice(h * (QC // 2), (h + 1) * (QC // 2))
                                    ps_h = psO.tile([P, QC // 2], F32, name="ps_h", tag="ps_o")
                                    for kb in range(NS):
                                        mm(ps_h, v_sb[kb][:, ec * P:(ec + 1) * P],
                                           pt[:, kb, hs], start=(kb == 0), stop=(kb == NS - 1))
                                    out_h = outp.tile([P, QC // 2], F32, name="out_h", tag="out_h")
                                    nc.vector.tensor_mul(out_h, ps_h, r_bc[:, hs])
                                    nc.sync.dma_start(
                                        out=outT_d[ec * P:(ec + 1) * P,
                                                   qc * QC + h * (QC // 2):qc * QC + (h + 1) * (QC // 2)],
                                        in_=out_h)
                                ps_os.append(None)
                            else:
                                ps_o = psO.tile([P, QC], F32, name="ps_o", tag="ps_o")
                                for kb in range(NS):
                                    mm(ps_o, v_sb[kb][:, ec * P:(ec + 1) * P], pt[:, kb, :],
                                       start=(kb == 0), stop=(kb == NS - 1))
                                ps_os.append(ps_o)
                            if ec == l1_after:
                                mm(l1, ones_col, h1[:, :, 0], start=True, stop=True)
                                nc.vector.reciprocal(out=r_sb, in_=l1)
                            if ec == psr_after:
                                mm(ps_r, ones_row, r_sb, start=True, stop=True)
                                nc.vector.tensor_copy(r_bc, ps_r)
                                for e2 in range(psr_after + 1):
                                    out_sb = outp.tile([P, QC], F32, name="out_sb", tag="out_sb")
                                    nc.vector.tensor_mul(out_sb, ps_os[e2], r_bc)
                                    nc.sync.dma_start(out=outT_d[e2 * P:(e2 + 1) * P, qs], in_=out_sb)
                                    ps_os[e2] = None
                        for ec in range(NEC):
                            if ps_os[ec] is None:
                                continue
                            out_sb = outp.tile([P, QC], F32, name="out_sb", tag="out_sb")
                            nc.vector.tensor_mul(out_sb, ps_os[ec], r_bc)
                            nc.sync.dma_start(out=outT_d[ec * P:(ec + 1) * P, qs], in_=out_sb)

    nc.compile()
    return nc


def _get_nc(mode):
    if mode not in _NC_CACHE:
        _NC_CACHE[mode] = _build(mode)
    return _NC_CACHE[mode]


def _prep_in_maps(x, Wq, Wk, Wv, mode):
    if mode == "bf16":
        import ml_dtypes

        def cast(a):
            return np.ascontiguousarray(a).astype(ml_dtypes.bfloat16)
    else:
        def cast(a):
            return np.ascontiguousarray(a, dtype=np.float32)

    scale = 1.0 / math.sqrt(DM)
    wq_h = cast((np.asarray(Wq, np.float32) * scale).T)
    wk_h = cast(np.asarray(Wk, np.float32).T)
    wv_h = cast(np.asarray(Wv, np.float32).T)
    x = np.asarray(x, np.float32)
    return [
        {"xt": cast(x[b].T), "wq": wq_h, "wk": wk_h, "wv": wv_h}
        for b in range(x.shape[0])
    ]


def _run(in_maps, mode=None, **kw):
    mode = mode or MODE
    nc = _get_nc(mode)
    return run_bass_kernel_spmd(nc, in_maps, core_ids=list(range(len(in_maps))), **kw)


def kernel(x, Wq, Wk, Wv):
    in_maps = _prep_in_maps(x, Wq, Wk, Wv, MODE)
    res = _run(in_maps)
    out = np.stack([r["outT"].T for r in res.results])
    return np.ascontiguousarray(out, dtype=np.float32)



# revision 2
# speedup vs baseline: 1.1526x; 1.1526x over previous
"""Trainium2 Bass kernel for nn_AttentionBlock (B=8, S=2048, D=512, f32).

Strategy: data-parallel over batch — one batch element per NeuronCore (8 cores,
same NEFF, SPMD). Per core, the full attention block is computed with the
"transposed scores" layout so no on-chip transposes are needed.

Key algebraic trick (merged QK): scores = (x Wq^T)(x Wk^T)^T / sqrt(D)
= x A x^T with A = Wq^T Wk / sqrt(D) precomputed on the host. This removes
one full projection (the k-projection) from the device: the scores matmul
contracts qaT = A^T x^T directly against xt, which doubles as the k-side
stationary operand.

  host prep:  xt = x[b].T            [D, S]
              wa = Wq^T Wk / sqrt(D) [D, D]   (d rows, e cols)
              wv = Wv.T              [D, D]
  stage A:    qaT[e, s] = sum_d wa[d, e] * xt[d, s]    (PSUM accum over d)
  stage B:    sT[k, q] = sum_e xt[e, k] * qaT[e, q]    (scores, transposed)
              p[k, q]  = exp(sT)     -- no max subtraction: scores in [-10, 10]
              h[q]     = running per-partition sum of p (one DVE add per kb)
  stage C:    v[s, e]  = sum_d xt[d, s] * wv[d, e]     (emitted inside qc=0)
              outT[e, q] = sum_k v[k, e] * p[k, q]
              outT *= 1/l  (ones-column matmul -> reciprocal -> K=1 broadcast)
  host post:  out[b] = outT.T

All tensors feeding the PE are bf16: bf16 keeps the matmul streaming rate of
f32r (1 col/cycle) but halves LDWEIGHTS time (97 ns vs 224 ns measured), which
un-hides-from/hides-under the 213 ns moving-operand stream — per-MM rate drops
from 272 ns (f32r) to the 216 ns floor. It also halves input DMA bytes.
Accumulation is fp32 in PSUM throughout; measured end-to-end rel err ~7e-3
(gate 2e-2).

Emission order is tuned so the PE never waits: ~24 wide (256-col) bf16 warmup
matmuls ramp the HAM clock while inputs DMA in (1-col warmups do NOT trip the
HAM activity monitor — measured); stage A runs s-chunk-major so the first
matmuls only need wa's first column block + the first xt chunk; the
v-projection fills the gap between scores(qc=0) and PV(qc=0); the denominator
is a running DVE add behind each exp so 1/l is ready one PV group after the
scores finish, letting the normalize+DMA of each output block start as soon
as its PV group completes (small kernel tail).
"""

import math

import numpy as np

import concourse.mybir as mybir
import concourse.tile as tile
from concourse import bacc
from concourse.bass_utils import run_bass_kernel_spmd

P = 128          # partitions
S = 2048         # sequence length
DM = 512         # d_model == d_attn == d_value
ND = DM // P     # 4  d-model chunks
NS = S // P      # 16 sequence blocks
QC = 512         # q-chunk width for fused score/PV stages
NQC = S // QC    # 4
NEC = DM // P    # 4  e-chunks of the output
N_WARMUP = 24    # wide PE warmup matmuls issued while input DMAs stream

F32 = mybir.dt.float32
F32R = mybir.dt.float32r
BF16 = mybir.dt.bfloat16

# 'bf16' (default): bf16 storage+matmuls.  'f32r': f32 storage, float32r matmuls.
MODE = "bf16"

_NC_CACHE = {}


def _build(mode):
    # tensors feeding the tensor engine carry the matmul dtype: the BIR
    # verifier requires fp32r matmul operands to be *produced* as float32r
    sb_dt = BF16 if mode == "bf16" else F32R
    # aux dtype for the l1 / broadcast matmul chain: bf16 runs those matmuls
    # single-pass; fp32 would be the 4-pass LOW/HIGH mode (~700 ns each)
    aux_dt = BF16 if mode == "bf16" else F32R
    nc = bacc.Bacc()

    xt_d = nc.dram_tensor("xt", [DM, S], sb_dt, kind="ExternalInput")
    wa_d = nc.dram_tensor("wa", [DM, DM], sb_dt, kind="ExternalInput")
    wv_d = nc.dram_tensor("wv", [DM, DM], sb_dt, kind="ExternalInput")
    outT_d = nc.dram_tensor("outT", [DM, S], F32, kind="ExternalOutput")

    mm = nc.tensor.matmul

    # low-precision outputs on DVE ops trip the guard; actual matmul
    # accumulation stays in fp32 PSUM throughout.
    with nc.allow_low_precision(reason="bf16/fp32r operand rounding; PSUM accumulation is fp32"), \
         tile.TileContext(nc) as tc:
        with tc.tile_pool(name="consts", bufs=1) as consts:
            # persistent SBUF tensors (distinct tags so nothing shares slots)
            wa_sb = [consts.tile([P, DM], sb_dt, name=f"wa{i}", tag=f"wa{i}") for i in range(ND)]
            wv_sb = [consts.tile([P, DM], sb_dt, name=f"wv{i}", tag=f"wv{i}") for i in range(ND)]
            xt_sb = [consts.tile([P, S], sb_dt, name=f"xt{i}", tag=f"xt{i}") for i in range(ND)]
            qt_sb = [consts.tile([P, S], sb_dt, name=f"qt{j}", tag=f"qt{j}") for j in range(ND)]
            v_sb = [consts.tile([P, DM], sb_dt, name=f"v{b}", tag=f"v{b}") for b in range(NS)]
            ones_col = consts.tile([P, 1], aux_dt, name="ones_col", tag="ones_col")
            ones_row = consts.tile([1, P], aux_dt, name="ones_row", tag="ones_row")
            # fp32 source for memset (memset can't write f32r) and the exp
            # table preload; warm_src feeds the wide warmup matmuls
            warm_raw = consts.tile([P, 256], F32, name="warm_raw", tag="warm_raw")
            warm_src = consts.tile([P, 256], sb_dt, name="warm_src", tag="warm_src")
            nc.vector.memset(warm_raw, 1.0)
            nc.vector.tensor_copy(warm_src, warm_raw)
            # preload the ACT Exp table during stage A — otherwise the first
            # exp of the scores stage pays the ~1.3us table load inline
            exp_warm = consts.tile([P, 1], F32, name="exp_warm", tag="exp_warm")
            nc.scalar.activation(out=exp_warm, in_=warm_raw[:, 0:1],
                                 func=mybir.ActivationFunctionType.Exp)
            ones_row_raw = consts.tile([1, P], F32, name="ones_row_raw", tag="ones_row_raw")
            nc.vector.memset(ones_row_raw, 1.0)
            nc.vector.tensor_copy(ones_col, warm_raw[:, 0:1])
            nc.vector.tensor_copy(ones_row, ones_row_raw)

            # input DMAs in first-use order: the first qaT psum group needs
            # only wa's j0 columns + the first xt chunk, so those go first
            for i in range(ND):
                nc.sync.dma_start(out=wa_sb[i][:, 0:P], in_=wa_d[i * P:(i + 1) * P, 0:P])
            for i in range(ND):
                nc.sync.dma_start(out=xt_sb[i][:, 0:QC], in_=xt_d[i * P:(i + 1) * P, 0:QC])
            for i in range(ND):
                nc.sync.dma_start(out=wa_sb[i][:, P:DM], in_=wa_d[i * P:(i + 1) * P, P:DM])
            for sc in range(1, NQC):
                for i in range(ND):
                    nc.sync.dma_start(
                        out=xt_sb[i][:, sc * QC:(sc + 1) * QC],
                        in_=xt_d[i * P:(i + 1) * P, sc * QC:(sc + 1) * QC],
                    )
            for i in range(ND):
                nc.sync.dma_start(out=wv_sb[i], in_=wv_d[i * P:(i + 1) * P, :])

            # ---- stage A: qa projection (s-chunk-major: the first groups
            # only need wa's first columns + the first xt chunk) --------------
            # psS/psM are opened while psA is still live so they get banks the
            # stage-A pool never touches and carry NO dependency on psA's
            # release (a pool release waits on ALL of the pool's accessors)
            from contextlib import ExitStack as _ExitStack
            with (
                tc.tile_pool(name="psS", bufs=4, space="PSUM") as psS,
                tc.tile_pool(name="psM", bufs=1, space="PSUM") as psM,
            ):
                _psa_stack = _ExitStack()
                psA = _psa_stack.enter_context(tc.tile_pool(name="psA", bufs=3, space="PSUM"))
                # PE warmup: wide matmuls with no data deps keep the PE array
                # genuinely busy while inputs stream in, so the HAM clock gate
                # opens (2.4 GHz) before real matmuls start. 1-col matmuls do
                # not register as PE activity for the HAM — these must be wide.
                warm = psA.tile([P, 256], F32, name="warm", tag="psA")
                for w in range(N_WARMUP):
                    mm(warm, warm_src[:, 0:P], warm_src, start=True, stop=True)
                # qaT copies alternate ACT/DVE: both are idle here, and
                # keeping ACT lightly loaded means the first exp of the
                # scores stage isn't queued behind stage-A copies
                for sc in range(NQC):
                    for j in range(ND):
                        ps = psA.tile([P, QC], F32, name="psA", tag="psA")
                        for i in range(ND):
                            mm(ps, wa_sb[i][:, j * P:(j + 1) * P],
                               xt_sb[i][:, sc * QC:(sc + 1) * QC],
                               start=(i == 0), stop=(i == ND - 1))
                        if j % 2 == 0:
                            nc.scalar.copy(qt_sb[j][:, sc * QC:(sc + 1) * QC], ps)
                        else:
                            nc.vector.tensor_copy(qt_sb[j][:, sc * QC:(sc + 1) * QC], ps)
                _psa_stack.close()

                # ---- stages B+C: scores -> exp -> denominators -> PV ------
                with (
                    tc.tile_pool(name="ptp", bufs=1) as ptp,
                    tc.tile_pool(name="work", bufs=2) as work,
                    tc.tile_pool(name="outp", bufs=3) as outp,
                    tc.tile_pool(name="psO", bufs=3, space="PSUM") as psO,
                ):
                    for qc in range(NQC):
                        qs = slice(qc * QC, (qc + 1) * QC)
                        pt = ptp.tile([P, NS, QC], sb_dt, name="pt", tag="pt")
                        # running per-partition sum of p on the (otherwise
                        # idle) DVE: one [P, QC] add right behind each exp, so
                        # the combined sum lands ~0.7us after the last exp and
                        # the l1 matmul can run right after the first PV group
                        h1 = work.tile([P, QC], F32, name="h1", tag="h1", bufs=1)
                        h1b = work.tile([P, QC], aux_dt, name="h1b", tag="h1b", bufs=1)
                        for kb in range(NS):
                            ps_s = psS.tile([P, QC], F32, name="ps_s", tag="ps_s")
                            for j in range(ND):
                                mm(ps_s, xt_sb[j][:, kb * P:(kb + 1) * P], qt_sb[j][:, qs],
                                   start=(j == 0), stop=(j == ND - 1))
                            nc.scalar.activation(out=pt[:, kb, :], in_=ps_s,
                                                 func=mybir.ActivationFunctionType.Exp)
                            if kb == 0:
                                nc.vector.tensor_copy(h1, pt[:, 0, :])
                            elif kb == NS - 1:
                                nc.vector.tensor_add(h1b, h1, pt[:, kb, :])
                            else:
                                nc.vector.tensor_add(h1, h1, pt[:, kb, :])

                        if qc == 0:
                            # v-projection, emitted here so it fills the PE while
                            # the qc=0 exps finish (PV(0) depends on all of them)
                            for b in range(NS):
                                psv = psO.tile([P, DM], F32, name="psv", tag="ps_o")
                                for i in range(ND):
                                    mm(psv, xt_sb[i][:, b * P:(b + 1) * P], wv_sb[i],
                                       start=(i == 0), stop=(i == ND - 1))
                                # scalar engine: the DVE is busy with the
                                # denominator adds here, and Tile's static
                                # schedule would run those first, starving PV(0)
                                nc.scalar.copy(v_sb[b], psv)

                        # PV: outT[e, q] = sum_k v[k, e] * p[k, q].  The l/1/l
                        # chain (l1 matmul -> DVE reciprocal -> K=1 broadcast
                        # matmul) is threaded through the PV groups so each step's
                        # input is ready just before the PE reaches it, and each
                        # output block is normalized + DMA'd as soon as possible.
                        l1 = psM.tile([1, QC], F32, name="l1", tag="lr")
                        r_sb = work.tile([1, QC], aux_dt, name="r_sb", tag="r_sb")
                        ps_r = psM.tile([P, QC], F32, name="ps_r", tag="lr")
                        r_bc = work.tile([P, QC], F32, name="r_bc", tag="r_bc")
                        last = qc == NQC - 1
                        ps_o0 = None
                        for ec in range(NEC):
                            if last and ec == NEC - 1:
                                # final output tile in two column halves (separate
                                # PSUM banks — a shared bank would serialize on the
                                # first half's norm read): the first half's
                                # norm+DMA overlap the second half's matmuls
                                for h in range(2):
                                    hs = slice(h * (QC // 2), (h + 1) * (QC // 2))
                                    ps_h = psO.tile([P, QC // 2], F32, name="ps_h", tag="ps_o")
                                    for kb in range(NS):
                                        mm(ps_h, v_sb[kb][:, ec * P:(ec + 1) * P],
                                           pt[:, kb, hs], start=(kb == 0), stop=(kb == NS - 1))
                                    out_h = outp.tile([P, QC // 2], F32, name="out_h", tag="out_h")
                                    nc.vector.tensor_mul(out_h, ps_h, r_bc[:, hs])
                                    nc.sync.dma_start(
                                        out=outT_d[ec * P:(ec + 1) * P,
                                                   qc * QC + h * (QC // 2):qc * QC + (h + 1) * (QC // 2)],
                                        in_=out_h)
                                continue
                            ps_o = psO.tile([P, QC], F32, name="ps_o", tag="ps_o")
                            for kb in range(NS):
                                mm(ps_o, v_sb[kb][:, ec * P:(ec + 1) * P], pt[:, kb, :],
                                   start=(kb == 0), stop=(kb == NS - 1))
                            if ec == 0:
                                # l -> 1/l after the first PV group (h1b is
                                # complete ~0.7us after the last exp)
                                mm(l1, ones_col, h1b, start=True, stop=True)
                                nc.vector.reciprocal(out=r_sb, in_=l1)
                                ps_o0 = ps_o
                            else:
                                if ec == 1:
                                    mm(ps_r, ones_row, r_sb, start=True, stop=True)
                                    nc.vector.tensor_copy(r_bc, ps_r)
                                    out_sb = outp.tile([P, QC], F32, name="out_sb", tag="out_sb")
                                    nc.vector.tensor_mul(out_sb, ps_o0, r_bc)
                                    nc.sync.dma_start(out=outT_d[0:P, qs], in_=out_sb)
                                    ps_o0 = None
                                out_sb = outp.tile([P, QC], F32, name="out_sb", tag="out_sb")
                                nc.vector.tensor_mul(out_sb, ps_o, r_bc)
                                nc.sync.dma_start(out=outT_d[ec * P:(ec + 1) * P, qs], in_=out_sb)

    nc.compile()
    return nc


def _get_nc(mode):
    if mode not in _NC_CACHE:
        _NC_CACHE[mode] = _build(mode)
    return _NC_CACHE[mode]


def _prep_in_maps(x, Wq, Wk, Wv, mode):
    if mode == "bf16":
        import ml_dtypes

        def cast(a):
            return np.ascontiguousarray(a).astype(ml_dtypes.bfloat16)
    else:
        def cast(a):
            return np.ascontiguousarray(a, dtype=np.float32)

    scale = 1.0 / math.sqrt(DM)
    # merged QK: scores = x (Wq^T Wk / sqrt(D)) x^T
    wa_h = cast((np.asarray(Wq, np.float64).T @ np.asarray(Wk, np.float64)
                 * scale).astype(np.float32))
    wv_h = cast(np.asarray(Wv, np.float32).T)
    x = np.asarray(x, np.float32)
    return [
        {"xt": cast(x[b].T), "wa": wa_h, "wv": wv_h}
        for b in range(x.shape[0])
    ]


def _run(in_maps, mode=None, **kw):
    mode = mode or MODE
    nc = _get_nc(mode)
    return run_bass_kernel_spmd(nc, in_maps, core_ids=list(range(len(in_maps))), **kw)


def kernel(x, Wq, Wk, Wv):
    in_maps = _prep_in_maps(x, Wq, Wk, Wv, MODE)
    res = _run(in_maps)
    out = np.stack([r["outT"].T for r in res.results])
    return np.ascontiguousarray(out, dtype=np.float32)


# revision 10
# speedup vs baseline: 1.1628x; 1.0088x over previous
"""Trainium2 Bass kernel for nn_AttentionBlock (B=8, S=2048, D=512, f32).

Strategy: data-parallel over batch — one batch element per NeuronCore (8 cores,
same NEFF, SPMD). Per core, the full attention block is computed with the
"transposed scores" layout so no on-chip transposes are needed.

Key algebraic trick (merged QK): scores = (x Wq^T)(x Wk^T)^T / sqrt(D)
= x A x^T with A = Wq^T Wk / sqrt(D) precomputed on the host. This removes
one full projection (the k-projection) from the device: the scores matmul
contracts qaT = A^T x^T directly against xt, which doubles as the k-side
stationary operand.

  host prep:  xt = x[b].T            [D, S]
              wa = Wq^T Wk / sqrt(D) [D, D]   (d rows, e cols)
              wv = Wv.T              [D, D]
  stage A:    qaT[e, s] = sum_d wa[d, e] * xt[d, s]    (PSUM accum over d)
  stage B:    sT[k, q] = sum_e xt[e, k] * qaT[e, q]    (scores, transposed)
              p[k, q]  = exp(sT)     -- no max subtraction: scores in [-10, 10]
              h[q]     = running per-partition sum of p (one DVE add per kb)
  stage C:    v[s, e]  = sum_d xt[d, s] * wv[d, e]     (emitted inside qc=0)
              outT[e, q] = sum_k v[k, e] * p[k, q]
              outT *= 1/l  (ones-column matmul -> reciprocal -> K=1 broadcast)
  host post:  out[b] = outT.T

All tensors feeding the PE are bf16: bf16 keeps the matmul streaming rate of
f32r (1 col/cycle) but halves LDWEIGHTS time (97 ns vs 224 ns measured), which
un-hides-from/hides-under the 213 ns moving-operand stream — per-MM rate drops
from 272 ns (f32r) to the 216 ns floor. It also halves input DMA bytes.
Accumulation is fp32 in PSUM throughout; measured end-to-end rel err ~7e-3
(gate 2e-2).

Emission order is tuned so the PE never waits: ~24 wide (256-col) bf16 warmup
matmuls ramp the HAM clock while inputs DMA in (1-col warmups do NOT trip the
HAM activity monitor — measured); stage A runs s-chunk-major so the first
matmuls only need wa's first column block + the first xt chunk; the
v-projection fills the gap between scores(qc=0) and PV(qc=0); the denominator
is a running DVE add behind each exp so 1/l is ready one PV group after the
scores finish, letting the normalize+DMA of each output block start as soon
as its PV group completes (small kernel tail).
"""

import math

import numpy as np

import concourse.mybir as mybir
import concourse.tile as tile
from concourse import bacc
from concourse.bass_utils import run_bass_kernel_spmd

P = 128          # partitions
S = 2048         # sequence length
DM = 512         # d_model == d_attn == d_value
ND = DM // P     # 4  d-model chunks
NS = S // P      # 16 sequence blocks
QC = 512         # q-chunk width for fused score/PV stages
NQC = S // QC    # 4
NEC = DM // P    # 4  e-chunks of the output
N_WARMUP = 20    # wide PE warmup matmuls issued while input DMAs stream

F32 = mybir.dt.float32
F32R = mybir.dt.float32r
BF16 = mybir.dt.bfloat16

# 'bf16' (default): bf16 storage+matmuls.  'f32r': f32 storage, float32r matmuls.
MODE = "bf16"

_NC_CACHE = {}


def _build(mode):
    # tensors feeding the tensor engine carry the matmul dtype: the BIR
    # verifier requires fp32r matmul operands to be *produced* as float32r
    sb_dt = BF16 if mode == "bf16" else F32R
    # aux dtype for the l1 / broadcast matmul chain: bf16 runs those matmuls
    # single-pass; fp32 would be the 4-pass LOW/HIGH mode (~700 ns each)
    aux_dt = BF16 if mode == "bf16" else F32R
    nc = bacc.Bacc()

    xt_d = nc.dram_tensor("xt", [DM, S], sb_dt, kind="ExternalInput")
    wa_d = nc.dram_tensor("wa", [DM, DM], sb_dt, kind="ExternalInput")
    wv_d = nc.dram_tensor("wv", [DM, DM], sb_dt, kind="ExternalInput")
    # output travels bf16 (halves the out-DMA; host upcasts to f32 — adds
    # ~0.4% worst-case to a ~5e-3 rel err, far under the 2e-2 gate)
    out_dt = BF16 if mode == "bf16" else F32
    outT_d = nc.dram_tensor("outT", [DM, S], out_dt, kind="ExternalOutput")

    mm = nc.tensor.matmul

    # low-precision outputs on DVE ops trip the guard; actual matmul
    # accumulation stays in fp32 PSUM throughout.
    with nc.allow_low_precision(reason="bf16/fp32r operand rounding; PSUM accumulation is fp32"), \
         tile.TileContext(nc) as tc:
        with tc.tile_pool(name="consts", bufs=1) as consts:
            # persistent SBUF tensors (distinct tags so nothing shares slots)
            wa_sb = [consts.tile([P, DM], sb_dt, name=f"wa{i}", tag=f"wa{i}") for i in range(ND)]
            wv_sb = [consts.tile([P, DM], sb_dt, name=f"wv{i}", tag=f"wv{i}") for i in range(ND)]
            xt_sb = [consts.tile([P, S], sb_dt, name=f"xt{i}", tag=f"xt{i}") for i in range(ND)]
            qt_sb = [consts.tile([P, S], sb_dt, name=f"qt{j}", tag=f"qt{j}") for j in range(ND)]
            v_sb = [consts.tile([P, DM], sb_dt, name=f"v{b}", tag=f"v{b}") for b in range(NS)]
            ones_col = consts.tile([P, 1], aux_dt, name="ones_col", tag="ones_col")
            ones_row = consts.tile([1, P], aux_dt, name="ones_row", tag="ones_row")
            # fp32 source for memset (memset can't write f32r) and the exp
            # table preload; warm_src feeds the wide warmup matmuls
            warm_raw = consts.tile([P, 256], F32, name="warm_raw", tag="warm_raw")
            warm_src = consts.tile([P, 256], sb_dt, name="warm_src", tag="warm_src")
            nc.vector.memset(warm_raw, 1.0)
            nc.vector.tensor_copy(warm_src, warm_raw)
            # preload the ACT Exp table during stage A — otherwise the first
            # exp of the scores stage pays the ~1.3us table load inline
            exp_warm = consts.tile([P, 1], F32, name="exp_warm", tag="exp_warm")
            nc.scalar.activation(out=exp_warm, in_=warm_raw[:, 0:1],
                                 func=mybir.ActivationFunctionType.Exp)
            ones_row_raw = consts.tile([1, P], F32, name="ones_row_raw", tag="ones_row_raw")
            nc.vector.memset(ones_row_raw, 1.0)
            nc.vector.tensor_copy(ones_col, warm_raw[:, 0:1])
            nc.vector.tensor_copy(ones_row, ones_row_raw)

            # input DMAs in first-use order: the first qaT psum group needs
            # only wa's j0 columns + the first xt chunk, so those go first
            for i in range(ND):
                nc.sync.dma_start(out=wa_sb[i][:, 0:P], in_=wa_d[i * P:(i + 1) * P, 0:P])
            for i in range(ND):
                nc.sync.dma_start(out=xt_sb[i][:, 0:QC], in_=xt_d[i * P:(i + 1) * P, 0:QC])
            for i in range(ND):
                nc.sync.dma_start(out=wa_sb[i][:, P:DM], in_=wa_d[i * P:(i + 1) * P, P:DM])
            # wv before the remaining xt chunks: the v-projection is
            # interleaved into stage A to soak up the xt DMA latency
            for i in range(ND):
                nc.sync.dma_start(out=wv_sb[i], in_=wv_d[i * P:(i + 1) * P, :])
            for sc in range(1, NQC):
                for i in range(ND):
                    nc.sync.dma_start(
                        out=xt_sb[i][:, sc * QC:(sc + 1) * QC],
                        in_=xt_d[i * P:(i + 1) * P, sc * QC:(sc + 1) * QC],
                    )

            # ---- stage A: qa projection (s-chunk-major: the first groups
            # only need wa's first columns + the first xt chunk) --------------
            # psS/psM are opened while psA is still live so they get banks the
            # stage-A pool never touches and carry NO dependency on psA's
            # release (a pool release waits on ALL of the pool's accessors)
            from contextlib import ExitStack as _ExitStack
            with (
                tc.tile_pool(name="psS", bufs=4, space="PSUM") as psS,
                tc.tile_pool(name="psM", bufs=1, space="PSUM") as psM,
            ):
                _psa_stack = _ExitStack()
                psA = _psa_stack.enter_context(tc.tile_pool(name="psA", bufs=3, space="PSUM"))
                # PE warmup: wide matmuls with no data deps keep the PE array
                # genuinely busy while inputs stream in, so the HAM clock gate
                # opens (2.4 GHz) before real matmuls start. 1-col matmuls do
                # not register as PE activity for the HAM — these must be wide.
                warm = psA.tile([P, 256], F32, name="warm", tag="psA")
                for w in range(N_WARMUP):
                    mm(warm, warm_src[:, 0:P], warm_src, start=True, stop=True)
                # The v-projection interleaves with the qa groups per s-chunk:
                # each 512-col xt chunk unlocks ~6.9us of matmuls (4 qa + 4 v
                # groups) against ~3.5us of DMA, so the PE rides out the xt
                # stream without stalling.  Copies alternate ACT/DVE: both are
                # idle here, and spreading them means the first exp of the
                # scores stage isn't queued behind a backlog of stage-A copies
                for sc in range(NQC):
                    for j in range(ND):
                        ps = psA.tile([P, QC], F32, name="psA", tag="psA")
                        for i in range(ND):
                            mm(ps, wa_sb[i][:, j * P:(j + 1) * P],
                               xt_sb[i][:, sc * QC:(sc + 1) * QC],
                               start=(i == 0), stop=(i == ND - 1))
                        if j % 2 == 0:
                            nc.scalar.copy(qt_sb[j][:, sc * QC:(sc + 1) * QC], ps)
                        else:
                            nc.vector.tensor_copy(qt_sb[j][:, sc * QC:(sc + 1) * QC], ps)
                    for b in range(4 * sc, 4 * sc + 4):
                        psv = psA.tile([P, DM], F32, name="psv", tag="psA")
                        for i in range(ND):
                            mm(psv, xt_sb[i][:, b * P:(b + 1) * P], wv_sb[i],
                               start=(i == 0), stop=(i == ND - 1))
                        if b % 2 == 0:
                            nc.scalar.copy(v_sb[b], psv)
                        else:
                            nc.vector.tensor_copy(v_sb[b], psv)
                _psa_stack.close()

                # ---- stages B+C: scores -> exp -> denominators -> PV ------
                with (
                    tc.tile_pool(name="ptp", bufs=1) as ptp,
                    tc.tile_pool(name="work", bufs=2) as work,
                    tc.tile_pool(name="outp", bufs=3) as outp,
                    tc.tile_pool(name="psO", bufs=3, space="PSUM") as psO,
                ):
                    for qc in range(NQC):
                        qs = slice(qc * QC, (qc + 1) * QC)
                        pt = ptp.tile([P, NS, QC], sb_dt, name="pt", tag="pt")
                        # running per-partition sum of p on the (otherwise
                        # idle) DVE: one [P, QC] add right behind each exp, so
                        # the combined sum lands ~0.7us after the last exp and
                        # the l1 matmul can run right after the first PV group
                        h1 = work.tile([P, QC], F32, name="h1", tag="h1", bufs=1)
                        h1b = work.tile([P, QC], aux_dt, name="h1b", tag="h1b", bufs=1)
                        for kb in range(NS):
                            ps_s = psS.tile([P, QC], F32, name="ps_s", tag="ps_s")
                            for j in range(ND):
                                mm(ps_s, xt_sb[j][:, kb * P:(kb + 1) * P], qt_sb[j][:, qs],
                                   start=(j == 0), stop=(j == ND - 1))
                            nc.scalar.activation(out=pt[:, kb, :], in_=ps_s,
                                                 func=mybir.ActivationFunctionType.Exp)
                            if kb == 0:
                                nc.vector.tensor_copy(h1, pt[:, 0, :])
                            elif kb == NS - 1:
                                nc.vector.tensor_add(h1b, h1, pt[:, kb, :])
                            else:
                                nc.vector.tensor_add(h1, h1, pt[:, kb, :])

                        # PV: outT[e, q] = sum_k v[k, e] * p[k, q].  The l/1/l
                        # chain (l1 matmul -> DVE reciprocal -> K=1 broadcast
                        # matmul) is threaded through the PV groups so each step's
                        # input is ready just before the PE reaches it, and each
                        # output block is normalized + DMA'd as soon as possible.
                        l1 = psM.tile([1, QC], F32, name="l1", tag="lr")
                        r_sb = work.tile([1, QC], aux_dt, name="r_sb", tag="r_sb")
                        ps_r = psM.tile([P, QC], F32, name="ps_r", tag="lr")
                        r_bc = work.tile([P, QC], F32, name="r_bc", tag="r_bc")
                        last = qc == NQC - 1
                        ps_o0 = None
                        for ec in range(NEC):
                            if last and ec == NEC - 1:
                                # final output tile in two column halves (separate
                                # PSUM banks — a shared bank would serialize on the
                                # first half's norm read): the first half's
                                # norm+DMA overlap the second half's matmuls
                                for h in range(2):
                                    hs = slice(h * (QC // 2), (h + 1) * (QC // 2))
                                    ps_h = psO.tile([P, QC // 2], F32, name="ps_h", tag="ps_o")
                                    for kb in range(NS):
                                        mm(ps_h, v_sb[kb][:, ec * P:(ec + 1) * P],
                                           pt[:, kb, hs], start=(kb == 0), stop=(kb == NS - 1))
                                    out_h = outp.tile([P, QC // 2], out_dt, name="out_h", tag="out_h")
                                    nc.vector.tensor_mul(out_h, ps_h, r_bc[:, hs])
                                    nc.sync.dma_start(
                                        out=outT_d[ec * P:(ec + 1) * P,
                                                   qc * QC + h * (QC // 2):qc * QC + (h + 1) * (QC // 2)],
                                        in_=out_h)
                                continue
                            ps_o = psO.tile([P, QC], F32, name="ps_o", tag="ps_o")
                            for kb in range(NS):
                                mm(ps_o, v_sb[kb][:, ec * P:(ec + 1) * P], pt[:, kb, :],
                                   start=(kb == 0), stop=(kb == NS - 1))
                            if ec == 0:
                                # l -> 1/l after the first PV group (h1b is
                                # complete ~0.7us after the last exp)
                                mm(l1, ones_col, h1b, start=True, stop=True)
                                nc.vector.reciprocal(out=r_sb, in_=l1)
                                ps_o0 = ps_o
                            else:
                                if ec == 1:
                                    mm(ps_r, ones_row, r_sb, start=True, stop=True)
                                    nc.vector.tensor_copy(r_bc, ps_r)
                                    out_sb = outp.tile([P, QC], out_dt, name="out_sb", tag="out_sb")
                                    nc.vector.tensor_mul(out_sb, ps_o0, r_bc)
                                    nc.sync.dma_start(out=outT_d[0:P, qs], in_=out_sb)
                                    ps_o0 = None
                                out_sb = outp.tile([P, QC], out_dt, name="out_sb", tag="out_sb")
                                nc.vector.tensor_mul(out_sb, ps_o, r_bc)
                                nc.sync.dma_start(out=outT_d[ec * P:(ec + 1) * P, qs], in_=out_sb)

    nc.compile()
    return nc


def _get_nc(mode):
    if mode not in _NC_CACHE:
        _NC_CACHE[mode] = _build(mode)
    return _NC_CACHE[mode]


def _prep_in_maps(x, Wq, Wk, Wv, mode):
    if mode == "bf16":
        import ml_dtypes

        def cast(a):
            return np.ascontiguousarray(a).astype(ml_dtypes.bfloat16)
    else:
        def cast(a):
            return np.ascontiguousarray(a, dtype=np.float32)

    scale = 1.0 / math.sqrt(DM)
    # merged QK: scores = x (Wq^T Wk / sqrt(D)) x^T
    wa_h = cast((np.asarray(Wq, np.float64).T @ np.asarray(Wk, np.float64)
                 * scale).astype(np.float32))
    wv_h = cast(np.asarray(Wv, np.float32).T)
    x = np.asarray(x, np.float32)
    return [
        {"xt": cast(x[b].T), "wa": wa_h, "wv": wv_h}
        for b in range(x.shape[0])
    ]


def _run(in_maps, mode=None, **kw):
    mode = mode or MODE
    nc = _get_nc(mode)
    return run_bass_kernel_spmd(nc, in_maps, core_ids=list(range(len(in_maps))), **kw)


def kernel(x, Wq, Wk, Wv):
    in_maps = _prep_in_maps(x, Wq, Wk, Wv, MODE)
    res = _run(in_maps)
    out = np.stack([np.asarray(r["outT"]).astype(np.float32).T for r in res.results])
    return np.ascontiguousarray(out, dtype=np.float32)


# revision 15
# speedup vs baseline: 1.1649x; 1.0019x over previous
"""Trainium2 Bass kernel for nn_AttentionBlock (B=8, S=2048, D=512, f32).

Strategy: data-parallel over batch — one batch element per NeuronCore (8 cores,
same NEFF, SPMD). Per core, the full attention block is computed with the
"transposed scores" layout so no on-chip transposes are needed.

Key algebraic trick (merged QK): scores = (x Wq^T)(x Wk^T)^T / sqrt(D)
= x A x^T with A = Wq^T Wk / sqrt(D) precomputed on the host. This removes
one full projection (the k-projection) from the device: the scores matmul
contracts qaT = A^T x^T directly against xt, which doubles as the k-side
stationary operand.

  host prep:  xt = x[b].T            [D, S]
              wa = Wq^T Wk / sqrt(D) [D, D]   (d rows, e cols)
              wv = Wv.T              [D, D]
  stage A:    qaT[e, s] = sum_d wa[d, e] * xt[d, s]    (PSUM accum over d)
              v[s, e]   = sum_d xt[d, s] * wv[d, e]    (interleaved per s-chunk)
  stage B:    sT[k, q] = sum_e xt[e, k] * qaT[e, q]    (scores, transposed)
              p[k, q]  = exp(sT)     -- no max subtraction: scores in [-10, 10]
              h[q]     = running per-partition sum of p (one DVE add per kb)
  stage C:    outT[e, q] = sum_k v[k, e] * p[k, q]
              outT *= 1/l  (GPSIMD partition all-reduce of h -> DVE reciprocal)
  host post:  out[b] = outT.T

All tensors feeding the PE are bf16: bf16 keeps the matmul streaming rate of
f32r (1 col/cycle) but halves LDWEIGHTS time (97 ns vs 224 ns measured), which
un-hides-from/hides-under the 213 ns moving-operand stream — per-MM rate drops
from 272 ns (f32r) to the 216 ns floor. It also halves input DMA bytes.
Accumulation is fp32 in PSUM throughout; measured end-to-end rel err ~7e-3
(gate 2e-2).

Emission order is tuned so the PE never waits: ~24 wide (256-col) bf16 warmup
matmuls ramp the HAM clock while inputs DMA in (1-col warmups do NOT trip the
HAM activity monitor — measured); stage A runs s-chunk-major so the first
matmuls only need wa's first column block + the first xt chunk; the
v-projection fills the gap between scores(qc=0) and PV(qc=0); the denominator
is a running DVE add behind each exp so 1/l is ready one PV group after the
scores finish, letting the normalize+DMA of each output block start as soon
as its PV group completes (small kernel tail).
"""

import math

import numpy as np

import concourse.bass_isa as bass_isa
import concourse.mybir as mybir
import concourse.tile as tile
from concourse import bacc
from concourse.bass_utils import run_bass_kernel_spmd

P = 128          # partitions
S = 2048         # sequence length
DM = 512         # d_model == d_attn == d_value
ND = DM // P     # 4  d-model chunks
NS = S // P      # 16 sequence blocks
QC = 512         # q-chunk width for fused score/PV stages
NQC = S // QC    # 4
NEC = DM // P    # 4  e-chunks of the output
N_WARMUP = 20    # wide PE warmup matmuls issued while input DMAs stream

F32 = mybir.dt.float32
F32R = mybir.dt.float32r
BF16 = mybir.dt.bfloat16

# 'bf16' (default): bf16 storage+matmuls.  'f32r': f32 storage, float32r matmuls.
MODE = "bf16"

_NC_CACHE = {}


def _build(mode):
    # tensors feeding the tensor engine carry the matmul dtype: the BIR
    # verifier requires fp32r matmul operands to be *produced* as float32r
    sb_dt = BF16 if mode == "bf16" else F32R
    # aux dtype for the l1 / broadcast matmul chain: bf16 runs those matmuls
    # single-pass; fp32 would be the 4-pass LOW/HIGH mode (~700 ns each)
    aux_dt = BF16 if mode == "bf16" else F32R
    nc = bacc.Bacc()

    xt_d = nc.dram_tensor("xt", [DM, S], sb_dt, kind="ExternalInput")
    wa_d = nc.dram_tensor("wa", [DM, DM], sb_dt, kind="ExternalInput")
    wv_d = nc.dram_tensor("wv", [DM, DM], sb_dt, kind="ExternalInput")
    # output travels bf16 (halves the out-DMA; host upcasts to f32 — adds
    # ~0.4% worst-case to a ~5e-3 rel err, far under the 2e-2 gate)
    out_dt = BF16 if mode == "bf16" else F32
    outT_d = nc.dram_tensor("outT", [DM, S], out_dt, kind="ExternalOutput")

    mm = nc.tensor.matmul

    # low-precision outputs on DVE ops trip the guard; actual matmul
    # accumulation stays in fp32 PSUM throughout.
    with nc.allow_low_precision(reason="bf16/fp32r operand rounding; PSUM accumulation is fp32"), \
         tile.TileContext(nc) as tc:
        with tc.tile_pool(name="consts", bufs=1) as consts:
            # persistent SBUF tensors (distinct tags so nothing shares slots)
            wa_sb = [consts.tile([P, DM], sb_dt, name=f"wa{i}", tag=f"wa{i}") for i in range(ND)]
            wv_sb = [consts.tile([P, DM], sb_dt, name=f"wv{i}", tag=f"wv{i}") for i in range(ND)]
            xt_sb = [consts.tile([P, S], sb_dt, name=f"xt{i}", tag=f"xt{i}") for i in range(ND)]
            qt_sb = [consts.tile([P, S], sb_dt, name=f"qt{j}", tag=f"qt{j}") for j in range(ND)]
            v_sb = [consts.tile([P, DM], sb_dt, name=f"v{b}", tag=f"v{b}") for b in range(NS)]
            # fp32 source for memset (memset can't write f32r) and the exp
            # table preload; warm_src feeds the wide warmup matmuls
            warm_raw = consts.tile([P, 256], F32, name="warm_raw", tag="warm_raw")
            warm_src = consts.tile([P, 256], sb_dt, name="warm_src", tag="warm_src")
            nc.vector.memset(warm_raw, 1.0)
            nc.vector.tensor_copy(warm_src, warm_raw)
            # preload the ACT Exp table during stage A — otherwise the first
            # exp of the scores stage pays the ~1.3us table load inline
            exp_warm = consts.tile([P, 1], F32, name="exp_warm", tag="exp_warm")
            nc.scalar.activation(out=exp_warm, in_=warm_raw[:, 0:1],
                                 func=mybir.ActivationFunctionType.Exp)

            # input DMAs in first-use order, as FEW triggers as gating allows:
            # each dma_start costs ~0.6us of issue time on the serial Sync
            # queue, so 28 small triggers would add ~17us of trigger latency.
            # xt goes in two column-halves per chunk (8 triggers): the first
            # halves gate stage A's sc=0/1 groups, the rest follow.
            for i in range(ND):
                nc.sync.dma_start(out=wa_sb[i][:, 0:P], in_=wa_d[i * P:(i + 1) * P, 0:P])
            for i in range(ND):
                nc.sync.dma_start(out=xt_sb[i][:, 0:2 * QC], in_=xt_d[i * P:(i + 1) * P, 0:2 * QC])
            for i in range(ND):
                nc.sync.dma_start(out=wa_sb[i][:, P:DM], in_=wa_d[i * P:(i + 1) * P, P:DM])
            # wv before the remaining xt chunks: the v-projection is
            # interleaved into stage A to soak up the xt DMA latency
            for i in range(ND):
                nc.sync.dma_start(out=wv_sb[i], in_=wv_d[i * P:(i + 1) * P, :])
            for i in range(ND):
                nc.sync.dma_start(
                    out=xt_sb[i][:, 2 * QC:S], in_=xt_d[i * P:(i + 1) * P, 2 * QC:S])

            # ---- stage A: qa projection (s-chunk-major: the first groups
            # only need wa's first columns + the first xt chunk) --------------
            # psS/psM are opened while psA is still live so they get banks the
            # stage-A pool never touches and carry NO dependency on psA's
            # release (a pool release waits on ALL of the pool's accessors)
            from contextlib import ExitStack as _ExitStack
            with (
                tc.tile_pool(name="psS", bufs=4, space="PSUM") as psS,
            ):
                _psa_stack = _ExitStack()
                psA = _psa_stack.enter_context(tc.tile_pool(name="psA", bufs=4, space="PSUM"))
                # PE warmup: wide matmuls with no data deps keep the PE array
                # genuinely busy while inputs stream in, so the HAM clock gate
                # opens (2.4 GHz) before real matmuls start. 1-col matmuls do
                # not register as PE activity for the HAM — these must be wide.
                warm = psA.tile([P, 256], F32, name="warm", tag="psA")
                for w in range(N_WARMUP):
                    mm(warm, warm_src[:, 0:P], warm_src, start=True, stop=True)
                # The v-projection interleaves with the qa groups per s-chunk:
                # each 512-col xt chunk unlocks ~6.9us of matmuls (4 qa + 4 v
                # groups) against ~3.5us of DMA, so the PE rides out the xt
                # stream without stalling.  Copies alternate ACT/DVE: both are
                # idle here, and spreading them means the first exp of the
                # scores stage isn't queued behind a backlog of stage-A copies
                for sc in range(NQC):
                    for j in range(ND):
                        ps = psA.tile([P, QC], F32, name="psA", tag="psA")
                        for i in range(ND):
                            mm(ps, wa_sb[i][:, j * P:(j + 1) * P],
                               xt_sb[i][:, sc * QC:(sc + 1) * QC],
                               start=(i == 0), stop=(i == ND - 1))
                        if j % 2 == 0:
                            nc.scalar.copy(qt_sb[j][:, sc * QC:(sc + 1) * QC], ps)
                        else:
                            nc.vector.tensor_copy(qt_sb[j][:, sc * QC:(sc + 1) * QC], ps)
                    for b in range(4 * sc, 4 * sc + 4):
                        psv = psA.tile([P, DM], F32, name="psv", tag="psA")
                        for i in range(ND):
                            mm(psv, xt_sb[i][:, b * P:(b + 1) * P], wv_sb[i],
                               start=(i == 0), stop=(i == ND - 1))
                        if b % 2 == 0:
                            nc.scalar.copy(v_sb[b], psv)
                        else:
                            nc.vector.tensor_copy(v_sb[b], psv)
                _psa_stack.close()

                # ---- stages B+C: scores -> exp -> denominators -> PV ------
                with (
                    tc.tile_pool(name="ptp", bufs=1) as ptp,
                    tc.tile_pool(name="work", bufs=2) as work,
                    tc.tile_pool(name="outp", bufs=3) as outp,
                    tc.tile_pool(name="psO", bufs=4, space="PSUM") as psO,
                ):
                    for qc in range(NQC):
                        qs = slice(qc * QC, (qc + 1) * QC)
                        pt = ptp.tile([P, NS, QC], sb_dt, name="pt", tag="pt")
                        # running per-partition sum of p on the (otherwise
                        # idle) DVE: one [P, QC] add right behind each exp, so
                        # the combined sum lands ~0.7us after the last exp
                        h1 = work.tile([P, QC], F32, name="h1", tag="h1", bufs=1)
                        for kb in range(NS):
                            ps_s = psS.tile([P, QC], F32, name="ps_s", tag="ps_s")
                            for j in range(ND):
                                mm(ps_s, xt_sb[j][:, kb * P:(kb + 1) * P], qt_sb[j][:, qs],
                                   start=(j == 0), stop=(j == ND - 1))
                            nc.scalar.activation(out=pt[:, kb, :], in_=ps_s,
                                                 func=mybir.ActivationFunctionType.Exp)
                            if kb == 0:
                                nc.vector.tensor_copy(h1, pt[:, 0, :])
                            else:
                                nc.vector.tensor_add(h1, h1, pt[:, kb, :])

                        # l -> 1/l entirely off the PE: a GPSIMD partition
                        # all-reduce fuses the cross-partition sum AND the
                        # broadcast (every partition gets l), then one DVE
                        # reciprocal yields the [P, QC] normalizer.  Both run
                        # on otherwise-idle engines during the first PV group.
                        l_bc = work.tile([P, QC], F32, name="l_bc", tag="l_bc", bufs=1)
                        r_bc = work.tile([P, QC], F32, name="r_bc", tag="r_bc")
                        nc.gpsimd.partition_all_reduce(l_bc, h1, 128, bass_isa.ReduceOp.add)
                        nc.vector.reciprocal(out=r_bc, in_=l_bc)

                        # PV: outT[e, q] = sum_k v[k, e] * p[k, q]; each output
                        # block is normalized + DMA'd as soon as its PV group
                        # completes, so the kernel tail is one block's norm+DMA
                        last = qc == NQC - 1
                        for ec in range(NEC):
                            if last and ec == NEC - 1:
                                # final output tile in two column halves (separate
                                # PSUM banks — a shared bank would serialize on the
                                # first half's norm read): the first half's
                                # norm+DMA overlap the second half's matmuls
                                for h in range(2):
                                    hs = slice(h * (QC // 2), (h + 1) * (QC // 2))
                                    ps_h = psO.tile([P, QC // 2], F32, name="ps_h", tag="ps_o")
                                    for kb in range(NS):
                                        mm(ps_h, v_sb[kb][:, ec * P:(ec + 1) * P],
                                           pt[:, kb, hs], start=(kb == 0), stop=(kb == NS - 1))
                                    out_h = outp.tile([P, QC // 2], out_dt, name="out_h", tag="out_h")
                                    nc.vector.tensor_mul(out_h, ps_h, r_bc[:, hs])
                                    nc.sync.dma_start(
                                        out=outT_d[ec * P:(ec + 1) * P,
                                                   qc * QC + h * (QC // 2):qc * QC + (h + 1) * (QC // 2)],
                                        in_=out_h)
                                continue
                            ps_o = psO.tile([P, QC], F32, name="ps_o", tag="ps_o")
                            for kb in range(NS):
                                mm(ps_o, v_sb[kb][:, ec * P:(ec + 1) * P], pt[:, kb, :],
                                   start=(kb == 0), stop=(kb == NS - 1))
                            out_sb = outp.tile([P, QC], out_dt, name="out_sb", tag="out_sb")
                            nc.vector.tensor_mul(out_sb, ps_o, r_bc)
                            nc.sync.dma_start(out=outT_d[ec * P:(ec + 1) * P, qs], in_=out_sb)

    nc.compile()
    return nc


def _get_nc(mode):
    if mode not in _NC_CACHE:
        _NC_CACHE[mode] = _build(mode)
    return _NC_CACHE[mode]


def _prep_in_maps(x, Wq, Wk, Wv, mode):
    if mode == "bf16":
        import ml_dtypes

        def cast(a):
            return np.ascontiguousarray(a).astype(ml_dtypes.bfloat16)
    else:
        def cast(a):
            return np.ascontiguousarray(a, dtype=np.float32)

    scale = 1.0 / math.sqrt(DM)
    # merged QK: scores = x (Wq^T Wk / sqrt(D)) x^T
    wa_h = cast((np.asarray(Wq, np.float64).T @ np.asarray(Wk, np.float64)
                 * scale).astype(np.float32))
    wv_h = cast(np.asarray(Wv, np.float32).T)
    x = np.asarray(x, np.float32)
    return [
        {"xt": cast(x[b].T), "wa": wa_h, "wv": wv_h}
        for b in range(x.shape[0])
    ]


def _run(in_maps, mode=None, **kw):
    mode = mode or MODE
    nc = _get_nc(mode)
    return run_bass_kernel_spmd(nc, in_maps, core_ids=list(range(len(in_maps))), **kw)


def kernel(x, Wq, Wk, Wv):
    in_maps = _prep_in_maps(x, Wq, Wk, Wv, MODE)
    res = _run(in_maps)
    out = np.stack([np.asarray(r["outT"]).astype(np.float32).T for r in res.results])
    return np.ascontiguousarray(out, dtype=np.float32)


# revision 20
# speedup vs baseline: 1.1893x; 1.0209x over previous
"""Trainium2 Bass kernel for nn_AttentionBlock (B=8, S=2048, D=512, f32).

Strategy: data-parallel over batch — one batch element per NeuronCore (8 cores,
same NEFF, SPMD). Per core, the full attention block is computed with the
"transposed scores" layout so no on-chip transposes are needed.

Key algebraic trick (merged QK): scores = (x Wq^T)(x Wk^T)^T / sqrt(D)
= x A x^T with A = Wq^T Wk / sqrt(D) precomputed on the host. This removes
one full projection (the k-projection) from the device: the scores matmul
contracts qaT = A^T x^T directly against xt, which doubles as the k-side
stationary operand.

  host prep:  xt = x[b].T            [D, S]
              wa = Wq^T Wk / sqrt(D) [D, D]   (d rows, e cols)
              wv = Wv.T              [D, D]
  stage A:    qaT[e, s] = sum_d wa[d, e] * xt[d, s]    (PSUM accum over d)
              v[s, e]   = sum_d xt[d, s] * wv[d, e]    (interleaved per s-chunk)
  stage B:    sT[k, q] = sum_e xt[e, k] * qaT[e, q]    (scores, transposed)
              p[k, q]  = exp(sT)     -- no max subtraction: scores in [-10, 10]
              h[q]     = running per-partition sum of p (one DVE add per kb)
  stage C:    outT[e, q] = sum_k v[k, e] * p[k, q]
              outT *= 1/l  (GPSIMD partition all-reduce of h -> DVE reciprocal)
  host post:  out[b] = outT.T

All tensors feeding the PE are bf16: bf16 keeps the matmul streaming rate of
f32r (1 col/cycle) but halves LDWEIGHTS time (97 ns vs 224 ns measured), which
un-hides-from/hides-under the 213 ns moving-operand stream — per-MM rate drops
from 272 ns (f32r) to the 216 ns floor. It also halves input DMA bytes.
Accumulation is fp32 in PSUM throughout; measured end-to-end rel err ~7e-3
(gate 2e-2).

Emission order is tuned so the PE never waits: ~24 wide (256-col) bf16 warmup
matmuls ramp the HAM clock while inputs DMA in (1-col warmups do NOT trip the
HAM activity monitor — measured); stage A runs s-chunk-major so the first
matmuls only need wa's first column block + the first xt chunk; the
v-projection fills the gap between scores(qc=0) and PV(qc=0); the denominator
is a running DVE add behind each exp so 1/l is ready one PV group after the
scores finish, letting the normalize+DMA of each output block start as soon
as its PV group completes (small kernel tail).
"""

import math

import numpy as np

import concourse.bass_isa as bass_isa
import concourse.mybir as mybir
import concourse.tile as tile
from concourse import bacc
from concourse.bass_utils import run_bass_kernel_spmd

P = 128          # partitions
S = 2048         # sequence length
DM = 512         # d_model == d_attn == d_value
ND = DM // P     # 4  d-model chunks
NS = S // P      # 16 sequence blocks
QC = 512         # q-chunk width for fused score/PV stages
NQC = S // QC    # 4
NEC = DM // P    # 4  e-chunks of the output
N_WARMUP = 40    # wide PE warmup matmuls issued while input DMAs stream

F32 = mybir.dt.float32
F32R = mybir.dt.float32r
BF16 = mybir.dt.bfloat16

# 'bf16' (default): bf16 storage+matmuls.  'f32r': f32 storage, float32r matmuls.
MODE = "bf16"

_NC_CACHE = {}


def _build(mode):
    # tensors feeding the tensor engine carry the matmul dtype: the BIR
    # verifier requires fp32r matmul operands to be *produced* as float32r
    sb_dt = BF16 if mode == "bf16" else F32R
    # aux dtype for the l1 / broadcast matmul chain: bf16 runs those matmuls
    # single-pass; fp32 would be the 4-pass LOW/HIGH mode (~700 ns each)
    aux_dt = BF16 if mode == "bf16" else F32R
    nc = bacc.Bacc()

    xt_d = nc.dram_tensor("xt", [DM, S], sb_dt, kind="ExternalInput")
    wa_d = nc.dram_tensor("wa", [DM, DM], sb_dt, kind="ExternalInput")
    wv_d = nc.dram_tensor("wv", [DM, DM], sb_dt, kind="ExternalInput")
    # output travels bf16 (halves the out-DMA; host upcasts to f32 — adds
    # ~0.4% worst-case to a ~5e-3 rel err, far under the 2e-2 gate)
    out_dt = BF16 if mode == "bf16" else F32
    outT_d = nc.dram_tensor("outT", [DM, S], out_dt, kind="ExternalOutput")

    mm = nc.tensor.matmul

    # low-precision outputs on DVE ops trip the guard; actual matmul
    # accumulation stays in fp32 PSUM throughout.
    with nc.allow_low_precision(reason="bf16/fp32r operand rounding; PSUM accumulation is fp32"), \
         tile.TileContext(nc) as tc:
        with tc.tile_pool(name="consts", bufs=1) as consts:
            # persistent SBUF tensors (distinct tags so nothing shares slots).
            # xt/wa/wv pack all d-chunks into ONE tile so each input needs a
            # single DMA trigger (dma_start costs ~0.6us of serial issue time
            # on the Sync queue; per-chunk triggers delayed the first stage-A
            # matmul by several us and let the HAM clock re-throttle)
            wa_all = consts.tile([P, ND, DM], sb_dt, name="wa", tag="wa")
            wv_all = consts.tile([P, ND, DM], sb_dt, name="wv", tag="wv")
            xt_all = consts.tile([P, ND, S], sb_dt, name="xt", tag="xt")
            qt_sb = [consts.tile([P, S], sb_dt, name=f"qt{j}", tag=f"qt{j}") for j in range(ND)]
            v_sb = [consts.tile([P, DM], sb_dt, name=f"v{b}", tag=f"v{b}") for b in range(NS)]
            # fp32 source for memset (memset can't write f32r) and the exp
            # table preload; warm_src feeds the wide warmup matmuls
            warm_raw = consts.tile([P, 256], F32, name="warm_raw", tag="warm_raw")
            warm_src = consts.tile([P, 256], sb_dt, name="warm_src", tag="warm_src")
            nc.vector.memset(warm_raw, 1.0)
            nc.vector.tensor_copy(warm_src, warm_raw)
            # preload the ACT Exp table during stage A — otherwise the first
            # exp of the scores stage pays the ~1.3us table load inline
            exp_warm = consts.tile([P, 1], F32, name="exp_warm", tag="exp_warm")
            nc.scalar.activation(out=exp_warm, in_=warm_raw[:, 0:1],
                                 func=mybir.ActivationFunctionType.Exp)

            # input DMAs in first-use order, FIVE triggers total (Tile tracks
            # sub-tile ranges, so stage-A groups gate on exactly the ranges
            # they read): wa j0-columns + the first xt half gate stage A's
            # sc=0/1 groups; wv lands before the interleaved v-projection
            xt_r = xt_d.rearrange("(i p) s -> p i s", p=P)
            wa_r = wa_d.rearrange("(i p) e -> p i e", p=P)
            wv_r = wv_d.rearrange("(i p) e -> p i e", p=P)
            nc.sync.dma_start(out=wa_all[:, :, 0:P], in_=wa_r[:, :, 0:P])
            nc.sync.dma_start(out=xt_all[:, :, 0:2 * QC], in_=xt_r[:, :, 0:2 * QC])
            nc.sync.dma_start(out=wa_all[:, :, P:DM], in_=wa_r[:, :, P:DM])
            nc.sync.dma_start(out=wv_all, in_=wv_r)
            nc.sync.dma_start(out=xt_all[:, :, 2 * QC:S], in_=xt_r[:, :, 2 * QC:S])

            # ---- stage A: qa projection (s-chunk-major: the first groups
            # only need wa's first columns + the first xt chunk) --------------
            # psS/psM are opened while psA is still live so they get banks the
            # stage-A pool never touches and carry NO dependency on psA's
            # release (a pool release waits on ALL of the pool's accessors)
            from contextlib import ExitStack as _ExitStack
            with (
                tc.tile_pool(name="psS", bufs=4, space="PSUM") as psS,
            ):
                _psa_stack = _ExitStack()
                psA = _psa_stack.enter_context(tc.tile_pool(name="psA", bufs=4, space="PSUM"))
                # PE warmup: wide matmuls with no data deps keep the PE array
                # genuinely busy while inputs stream in, so the HAM clock gate
                # opens (2.4 GHz) before real matmuls start. 1-col matmuls do
                # not register as PE activity for the HAM — these must be wide.
                warm = psA.tile([P, 256], F32, name="warm", tag="psA")
                for w in range(N_WARMUP):
                    mm(warm, warm_src[:, 0:P], warm_src, start=True, stop=True)
                # The v-projection interleaves with the qa groups per s-chunk:
                # each 512-col xt chunk unlocks ~6.9us of matmuls (4 qa + 4 v
                # groups) against ~3.5us of DMA, so the PE rides out the xt
                # stream without stalling.  Copies alternate ACT/DVE: both are
                # idle here, and spreading them means the first exp of the
                # scores stage isn't queued behind a backlog of stage-A copies
                for sc in range(NQC):
                    for j in range(ND):
                        ps = psA.tile([P, QC], F32, name="psA", tag="psA")
                        for i in range(ND):
                            mm(ps, wa_all[:, i, j * P:(j + 1) * P],
                               xt_all[:, i, sc * QC:(sc + 1) * QC],
                               start=(i == 0), stop=(i == ND - 1))
                        if j % 2 == 0:
                            nc.scalar.copy(qt_sb[j][:, sc * QC:(sc + 1) * QC], ps)
                        else:
                            nc.vector.tensor_copy(qt_sb[j][:, sc * QC:(sc + 1) * QC], ps)
                    for b in range(4 * sc, 4 * sc + 4):
                        psv = psA.tile([P, DM], F32, name="psv", tag="psA")
                        for i in range(ND):
                            mm(psv, xt_all[:, i, b * P:(b + 1) * P], wv_all[:, i, :],
                               start=(i == 0), stop=(i == ND - 1))
                        if b % 2 == 0:
                            nc.scalar.copy(v_sb[b], psv)
                        else:
                            nc.vector.tensor_copy(v_sb[b], psv)
                _psa_stack.close()

                # ---- stages B+C: scores -> exp -> denominators -> PV ------
                with (
                    tc.tile_pool(name="ptp", bufs=1) as ptp,
                    tc.tile_pool(name="work", bufs=2) as work,
                    tc.tile_pool(name="outp", bufs=3) as outp,
                    tc.tile_pool(name="psO", bufs=4, space="PSUM") as psO,
                ):
                    for qc in range(NQC):
                        qs = slice(qc * QC, (qc + 1) * QC)
                        pt = ptp.tile([P, NS, QC], sb_dt, name="pt", tag="pt")
                        # running per-partition sum of p on the (otherwise
                        # idle) DVE: one [P, QC] add right behind each exp, so
                        # the combined sum lands ~0.7us after the last exp
                        h1 = work.tile([P, QC], F32, name="h1", tag="h1", bufs=1)
                        for kb in range(NS):
                            ps_s = psS.tile([P, QC], F32, name="ps_s", tag="ps_s")
                            for j in range(ND):
                                mm(ps_s, xt_all[:, j, kb * P:(kb + 1) * P], qt_sb[j][:, qs],
                                   start=(j == 0), stop=(j == ND - 1))
                            nc.scalar.activation(out=pt[:, kb, :], in_=ps_s,
                                                 func=mybir.ActivationFunctionType.Exp)
                            if kb == 0:
                                nc.vector.tensor_copy(h1, pt[:, 0, :])
                            else:
                                nc.vector.tensor_add(h1, h1, pt[:, kb, :])

                        # l -> 1/l entirely off the PE: a GPSIMD partition
                        # all-reduce fuses the cross-partition sum AND the
                        # broadcast (every partition gets l), then one DVE
                        # reciprocal yields the [P, QC] normalizer.  Both run
                        # on otherwise-idle engines during the first PV group.
                        l_bc = work.tile([P, QC], F32, name="l_bc", tag="l_bc", bufs=1)
                        r_bc = work.tile([P, QC], F32, name="r_bc", tag="r_bc")
                        nc.gpsimd.partition_all_reduce(l_bc, h1, 128, bass_isa.ReduceOp.add)
                        nc.vector.reciprocal(out=r_bc, in_=l_bc)

                        # PV: outT[e, q] = sum_k v[k, e] * p[k, q]; each output
                        # block is normalized + DMA'd as soon as its PV group
                        # completes, so the kernel tail is one block's norm+DMA
                        last = qc == NQC - 1
                        for ec in range(NEC):
                            if last and ec == NEC - 1:
                                # final output tile in two column halves (separate
                                # PSUM banks — a shared bank would serialize on the
                                # first half's norm read): the first half's
                                # norm+DMA overlap the second half's matmuls
                                for h in range(2):
                                    hs = slice(h * (QC // 2), (h + 1) * (QC // 2))
                                    ps_h = psO.tile([P, QC // 2], F32, name="ps_h", tag="ps_o")
                                    for kb in range(NS):
                                        mm(ps_h, v_sb[kb][:, ec * P:(ec + 1) * P],
                                           pt[:, kb, hs], start=(kb == 0), stop=(kb == NS - 1))
                                    out_h = outp.tile([P, QC // 2], out_dt, name="out_h", tag="out_h")
                                    nc.vector.tensor_mul(out_h, ps_h, r_bc[:, hs])
                                    nc.sync.dma_start(
                                        out=outT_d[ec * P:(ec + 1) * P,
                                                   qc * QC + h * (QC // 2):qc * QC + (h + 1) * (QC // 2)],
                                        in_=out_h)
                                continue
                            ps_o = psO.tile([P, QC], F32, name="ps_o", tag="ps_o")
                            for kb in range(NS):
                                mm(ps_o, v_sb[kb][:, ec * P:(ec + 1) * P], pt[:, kb, :],
                                   start=(kb == 0), stop=(kb == NS - 1))
                            out_sb = outp.tile([P, QC], out_dt, name="out_sb", tag="out_sb")
                            nc.vector.tensor_mul(out_sb, ps_o, r_bc)
                            nc.sync.dma_start(out=outT_d[ec * P:(ec + 1) * P, qs], in_=out_sb)

    nc.compile()
    return nc


def _get_nc(mode):
    if mode not in _NC_CACHE:
        _NC_CACHE[mode] = _build(mode)
    return _NC_CACHE[mode]


def _prep_in_maps(x, Wq, Wk, Wv, mode):
    if mode == "bf16":
        import ml_dtypes

        def cast(a):
            return np.ascontiguousarray(a).astype(ml_dtypes.bfloat16)
    else:
        def cast(a):
            return np.ascontiguousarray(a, dtype=np.float32)

    scale = 1.0 / math.sqrt(DM)
    # merged QK: scores = x (Wq^T Wk / sqrt(D)) x^T
    wa_h = cast((np.asarray(Wq, np.float64).T @ np.asarray(Wk, np.float64)
                 * scale).astype(np.float32))
    wv_h = cast(np.asarray(Wv, np.float32).T)
    x = np.asarray(x, np.float32)
    return [
        {"xt": cast(x[b].T), "wa": wa_h, "wv": wv_h}
        for b in range(x.shape[0])
    ]


def _run(in_maps, mode=None, **kw):
    mode = mode or MODE
    nc = _get_nc(mode)
    return run_bass_kernel_spmd(nc, in_maps, core_ids=list(range(len(in_maps))), **kw)


def kernel(x, Wq, Wk, Wv):
    in_maps = _prep_in_maps(x, Wq, Wk, Wv, MODE)
    res = _run(in_maps)
    out = np.stack([np.asarray(r["outT"]).astype(np.float32).T for r in res.results])
    return np.ascontiguousarray(out, dtype=np.float32)
